# revision 5
# baseline (speedup 1.0000x reference)
"""Fused attention-block kernel for Trainium2, 8-core data-parallel over batch.

Computation (see harness reference): three BN+ReLU linear branches from the
same input, attention (QK^T/16 -> softmax -> AV), then a fourth BN+ReLU
linear.  BatchNorm1d is training-mode per-channel over (batch, feature) with
channel = sequence position, so batch-sharding needs a cross-core stats
all-reduce (sync-BN); weights are replicated.

Hardcoded: B=256, N=256, D=256, 8 cores -> 32 batches (8192 tokens) per core.
"""
import sys
import types

sys.path.insert(0, "/opt/trn_rl_repo")

import numpy as np
import ml_dtypes
from contextlib import ExitStack

import concourse.bass as bass
import concourse.mybir as mybir
import concourse.tile as tile
from concourse.masks import make_identity

BF16 = mybir.dt.bfloat16
F32 = mybir.dt.float32
NCORES = 8
B_LOC = 32          # batches per core
T = B_LOC * 256     # tokens per core
EPS = 1e-5


def _install_profile_shim():
    """run_bass_kernel_spmd(trace=True) under axon needs antenv.axon_hooks,
    which this image lacks; synthesize it (harmless if tracing unused)."""
    if "antenv.axon_hooks" in sys.modules:
        return
    try:
        import antenv
        mod = types.ModuleType("antenv.axon_hooks")
        mod._hook = None
        mod.set_axon_ntff_profile_hook = lambda h: setattr(mod, "_hook", h)
        mod.get_axon_ntff_profile_hook = lambda: mod._hook
        sys.modules["antenv.axon_hooks"] = mod
        antenv.axon_hooks = mod
        from trn_agent_boot.trn_boot import _ntff_profile_via_ctypes
        hook = _ntff_profile_via_ctypes("/opt/axon/libaxon_pjrt.so")
        if hook is not None:
            mod.set_axon_ntff_profile_hook(hook)
    except Exception:
        pass


def _legalize_waits(nc, max_waits=1):
    """HW instructions carry one sync-wait slot; walrus rejects instructions
    with too many waits.  Hoist extras onto engine-matched NoOps."""
    for f in nc.m.functions:
        for bb in f.blocks:
            insts = bb.instructions
            new_list = []
            for inst in insts:
                si = inst.sync_info
                if si is not None and len(si.on_wait) > max_waits:
                    waits = list(si.on_wait)
                    extra, keep = waits[:-max_waits], waits[-max_waits:]
                    for j, w in enumerate(extra):
                        nop = mybir.InstNoOp(
                            name=f"{inst.name}-waitnop{j}",
                            engine=inst.engine,
                            ins=[], outs=[],
                            sync_info=mybir.SyncInfo(on_wait=[w], on_update=[]),
                        )
                        nc.register_instruction(nop, overwrite=True)
                        new_list.append(nop)
                    inst.sync_info = mybir.SyncInfo(
                        on_wait=keep, on_update=list(si.on_update))
                new_list.append(inst)
            del insts[:]
            for x in new_list:
                insts.append(x)


def build_program(use_collectives=True, pool_compute=False):
    nc = bass.Bass("TRN2", target_bir_lowering=False, debug=False,
                   num_devices=NCORES)
    GP_COMPUTE = nc.gpsimd if pool_compute else nc.vector

    x_d = nc.dram_tensor("x", [T, 256], F32, kind="ExternalInput")
    w123_d = nc.dram_tensor("w123", [128, 2, 768], BF16, kind="ExternalInput")
    w4_d = nc.dram_tensor("w4", [128, 2, 260], BF16, kind="ExternalInput")
    bb_d = nc.dram_tensor("bb", [128, 4, 256], F32, kind="ExternalInput")
    gb_d = nc.dram_tensor("gb", [128, 2, 2], F32, kind="ExternalInput")
    hc_d = nc.dram_tensor("hc", [128, 8], F32, kind="ExternalInput")
    out_d = nc.dram_tensor("out", [T, 256], F32, kind="ExternalOutput")

    groups = [list(range(NCORES))]
    x_r = x_d.ap().rearrange("(b h p) e -> p b h e", b=B_LOC, h=2, p=128)
    out_r = out_d.ap().rearrange("(b h p) e -> p b h e", b=B_LOC, h=2, p=128)

    with ExitStack() as ctx:
        tc = ctx.enter_context(tile.TileContext(nc))
        big = ctx.enter_context(tc.tile_pool(name="big", bufs=1))
        small = ctx.enter_context(tc.tile_pool(name="small", bufs=1))
        stage = ctx.enter_context(tc.tile_pool(name="stage", bufs=3))
        ps = ctx.enter_context(tc.tile_pool(name="ps", bufs=8, space="PSUM"))
        dram = ctx.enter_context(tc.tile_pool(name="dram", bufs=1, space="DRAM"))

        # ---- constant loads -------------------------------------------------
        w123 = small.tile([128, 2, 768], BF16, tag="w123")
        w4 = small.tile([128, 2, 260], BF16, tag="w4")
        bbt = small.tile([128, 4, 256], F32, tag="bbt")
        gbt = small.tile([128, 2, 2], F32, tag="gbt")
        hct = small.tile([128, 8], F32, tag="hct")
        idn = small.tile([128, 128], BF16, tag="idn")
        nc.sync.dma_start(out=w123[:], in_=w123_d.ap())
        nc.sync.dma_start(out=w4[:], in_=w4_d.ap())
        nc.sync.dma_start(out=bbt[:], in_=bb_d.ap())
        nc.sync.dma_start(out=gbt[:], in_=gb_d.ap())
        nc.sync.dma_start(out=hct[:], in_=hc_d.ap())
        make_identity(nc, idn[:])

        # ---- load x, cast bf16, transpose to xT[d, token] -------------------
        xT = big.tile([128, 2, T], BF16, tag="tp1")          # (dchunk, token)
        for b in range(B_LOC):
            xin = stage.tile([128, 2, 256], F32, tag="xin")
            nc.sync.dma_start(out=xin[:], in_=x_r[:, b, :, :])
            xbf = stage.tile([128, 2, 256], BF16, tag="xbf")
            GP_COMPUTE.tensor_copy(out=xbf[:], in_=xin[:])
            pst = ps.tile([128, 2, 2, 128], BF16, tag="ps")  # (dc, h, t)
            for h in range(2):
                for dc in range(2):
                    nc.tensor.transpose(
                        out=pst[:, dc, h, :],
                        in_=xbf[:, h, dc * 128:(dc + 1) * 128],
                        identity=idn[:])
            nc.any.tensor_copy(
                out=xT[:, :, b * 256:(b + 1) * 256],
                in_=pst[:].rearrange("p dc h t -> p dc (h t)"))

        # ---- helper: per-layer BN scale/shift from all-reduced stats --------
        def bn_finalize(lidx, artot, wterm=None):
            """artot [128,4]: per half h: (sum of core means, sum of core
            E[y^2]) at cols 2h, 2h+1.  Returns (s, bst) tiles."""
            meany = small.tile([128, 2], F32, tag=f"meany{lidx}")
            ey2 = small.tile([128, 2], F32, tag=f"ey2{lidx}")
            nc.vector.tensor_scalar_mul(meany[:], artot[:, 0:4:2], 1.0 / NCORES)
            nc.vector.tensor_scalar_mul(ey2[:], artot[:, 1:4:2], 1.0 / NCORES)
            meanz = small.tile([128, 2], F32, tag=f"meanz{lidx}")
            nc.vector.tensor_scalar_add(meanz[:], meany[:], hct[:, lidx:lidx + 1])
            if wterm is not None:
                # exact E[z^2] = E[y^2] + 2*E[y b] + mean(b^2)
                eyb = small.tile([128, 2], F32, tag=f"eyb{lidx}")
                nc.vector.tensor_scalar_mul(eyb[:], wterm[:], 2.0 / 65536.0)
                nc.vector.tensor_tensor(out=ey2[:], in0=ey2[:], in1=eyb[:],
                                        op=mybir.AluOpType.add)
                nc.vector.tensor_scalar_add(ey2[:], ey2[:], hct[:, 7:8])
                varz = small.tile([128, 2], F32, tag=f"varz{lidx}")
                m2 = small.tile([128, 2], F32, tag=f"m2_{lidx}")
                nc.vector.tensor_tensor(out=m2[:], in0=meanz[:], in1=meanz[:],
                                        op=mybir.AluOpType.mult)
                nc.vector.tensor_tensor(out=varz[:], in0=ey2[:], in1=m2[:],
                                        op=mybir.AluOpType.subtract)
                nc.vector.tensor_scalar_add(varz[:], varz[:], EPS)
            else:
                # var_z ~= var_y + var(b) (bias-covariance negligible here)
                vary = small.tile([128, 2], F32, tag=f"vary{lidx}")
                m2 = small.tile([128, 2], F32, tag=f"m2_{lidx}")
                nc.vector.tensor_tensor(out=m2[:], in0=meany[:], in1=meany[:],
                                        op=mybir.AluOpType.mult)
                nc.vector.tensor_tensor(out=vary[:], in0=ey2[:], in1=m2[:],
                                        op=mybir.AluOpType.subtract)
                varz = small.tile([128, 2], F32, tag=f"varz{lidx}")
                nc.vector.tensor_scalar(varz[:], vary[:],
                                        hct[:, 4 + lidx:5 + lidx], EPS,
                                        mybir.AluOpType.add,
                                        mybir.AluOpType.add)
            sd = small.tile([128, 2], F32, tag=f"sd{lidx}")
            nc.scalar.sqrt(out=sd[:], in_=varz[:])
            rstd = small.tile([128, 2], F32, tag=f"rstd{lidx}")
            nc.vector.reciprocal(out=rstd[:], in_=sd[:])
            s = small.tile([128, 2], F32, tag=f"s{lidx}")
            nc.vector.tensor_tensor(out=s[:], in0=rstd[:], in1=gbt[:, :, 0],
                                    op=mybir.AluOpType.mult)
            tsh = small.tile([128, 2], F32, tag=f"tsh{lidx}")
            nc.vector.tensor_tensor(out=tsh[:], in0=meanz[:], in1=s[:],
                                    op=mybir.AluOpType.mult)
            nc.vector.tensor_tensor(out=tsh[:], in0=gbt[:, :, 1], in1=tsh[:],
                                    op=mybir.AluOpType.subtract)
            bst = small.tile([128, 2, 256], F32, tag=f"bst{lidx}")
            for h in range(2):
                nc.vector.tensor_scalar(bst[:, h, :], bbt[:, lidx, :],
                                        s[:, h:h + 1], tsh[:, h:h + 1],
                                        mybir.AluOpType.mult,
                                        mybir.AluOpType.add)
            return s, bst

        def emit_allreduce(lidx, arin, width):
            ar_i = dram.tile([128, width], F32, tag=f"ari{lidx}")
            ar_o = dram.tile([128, width], F32, tag=f"aro{lidx}")
            nc.sync.dma_start(out=ar_i[:], in_=arin[:])
            if use_collectives:
                nc.gpsimd.collective_compute(
                    "AllReduce", mybir.AluOpType.add, replica_groups=groups,
                    ins=[ar_i[:].opt()], outs=[ar_o[:].opt()])
            else:
                nc.gpsimd.dma_start(out=ar_o[:], in_=ar_i[:])
            artot = small.tile([128, width], F32, tag=f"artot{lidx}")
            nc.sync.dma_start(out=artot[:], in_=ar_o[:])
            return artot

        def stats_to_arin(lidx, stats, width=4):
            """stats[h]: [128, B_LOC, 6] bn_stats rows -> arin [128, width]
            cols 2h=(core mean), 2h+1=(core E[y^2])."""
            arin = small.tile([128, width], F32, tag=f"arin{lidx}")
            for h in range(2):
                mv = small.tile([128, 2], F32, tag=f"mv{lidx}_{h}")
                nc.vector.bn_aggr(out=mv[:], in_=stats[h][:])
                nc.vector.tensor_copy(out=arin[:, 2 * h:2 * h + 1], in_=mv[:, 0:1])
                m2 = small.tile([128, 1], F32, tag=f"am2{lidx}_{h}")
                nc.vector.tensor_tensor(out=m2[:], in0=mv[:, 0:1], in1=mv[:, 0:1],
                                        op=mybir.AluOpType.mult)
                nc.vector.tensor_tensor(out=arin[:, 2 * h + 1:2 * h + 2],
                                        in0=mv[:, 1:2], in1=m2[:],
                                        op=mybir.AluOpType.add)
            return arin

        # ---- layers 1-3: matmul + stats, layer-sequential so each layer's ---
        # ---- stats all-reduce overlaps the next layer's matmuls            --
        z_tags = ["tpA", "tpB", "tpC"]
        z_sb = []
        s_l, bst_l = [None] * 3, [None] * 3
        artots = []
        for l in range(3):
            z = big.tile([128, B_LOC, 2, 256], BF16, tag=z_tags[l])
            z_sb.append(z)
            stats = [small.tile([128, B_LOC, 6], F32, tag=f"st{l}_{h}",
                                name=f"st{l}_{h}") for h in range(2)]
            for b in range(B_LOC):
                psz = ps.tile([128, 2, 256], F32, tag="ps")
                for h in range(2):
                    for dc in range(2):
                        nc.tensor.matmul(
                            out=psz[:, h, :],
                            lhsT=xT[:, dc, b * 256 + h * 128: b * 256 + (h + 1) * 128],
                            rhs=w123[:, dc, l * 256:(l + 1) * 256],
                            start=(dc == 0), stop=(dc == 1))
                for h in range(2):
                    nc.vector.bn_stats(out=stats[h][:, b, :], in_=psz[:, h, :])
                nc.any.tensor_copy(out=z[:, b, :, :], in_=psz[:])
            arin = stats_to_arin(l, stats)
            artots.append(emit_allreduce(l, arin, 4))

        for l in range(3):
            s_l[l], bst_l[l] = bn_finalize(l, artots[l])

        # ---- apply BN+ReLU; x1,x2 transposed (relu fused into psum copy), --
        # ---- x3 kept token-major with an all-ones column for softmax sums  --
        x1T = big.tile([128, 2, T], BF16, tag="tp1")
        x2T = big.tile([128, 2, T], BF16, tag="tpA")
        x3a = big.tile([128, B_LOC, 2, 260], BF16, tag="tpB")
        nc.vector.memset(x3a[:, :, :, 256:257], 1.0)
        for l, xiT in ((0, x1T), (1, x2T)):
            for b in range(B_LOC):
                stg = stage.tile([128, 2, 256], BF16, tag=f"app{l}")
                for h in range(2):
                    nc.vector.scalar_tensor_tensor(
                        out=stg[:, h, :], in0=z_sb[l][:, b, h, :],
                        scalar=s_l[l][:, h:h + 1], in1=bst_l[l][:, h, :],
                        op0=mybir.AluOpType.mult, op1=mybir.AluOpType.add)
                pst = ps.tile([128, 2, 2, 128], BF16, tag="ps")
                for h in range(2):
                    for dc in range(2):
                        nc.tensor.transpose(
                            out=pst[:, dc, h, :],
                            in_=stg[:, h, dc * 128:(dc + 1) * 128],
                            identity=idn[:])
                nc.scalar.activation(
                    out=xiT[:, :, b * 256:(b + 1) * 256],
                    in_=pst[:].rearrange("p dc h t -> p dc (h t)"),
                    func=mybir.ActivationFunctionType.Relu)
        for b in range(B_LOC):
            stg = stage.tile([128, 2, 256], BF16, tag="app2")
            for h in range(2):
                nc.vector.scalar_tensor_tensor(
                    out=stg[:, h, :], in0=z_sb[2][:, b, h, :],
                    scalar=s_l[2][:, h:h + 1], in1=bst_l[2][:, h, :],
                    op0=mybir.AluOpType.mult, op1=mybir.AluOpType.add)
            for h in range(2):
                GP_COMPUTE.tensor_scalar_max(x3a[:, b, h, 0:256], stg[:, h, :], 0.0)

        # ---- attention + layer 4 -------------------------------------------
        z4 = big.tile([128, B_LOC, 2, 256], BF16, tag="tpC")
        stats4 = [small.tile([128, B_LOC, 6], F32, tag=f"st4_{h}",
                             name=f"st4_{h}") for h in range(2)]
        wsums = small.tile([128, 2, B_LOC], F32, tag="wsums")
        for b in range(B_LOC):
            # S^T[m, n] per batch (exp via ACT, no max-subtraction: logits<=~7)
            pss = ps.tile([128, 2, 256], F32, tag="ps")      # (mchunk, n)
            for mc in range(2):
                for ec in range(2):
                    nc.tensor.matmul(
                        out=pss[:, mc, :],
                        lhsT=x2T[:, ec, b * 256 + mc * 128: b * 256 + (mc + 1) * 128],
                        rhs=x1T[:, ec, b * 256:(b + 1) * 256],
                        start=(ec == 0), stop=(ec == 1))
            pt = stage.tile([128, 2, 256], BF16, tag="pt")   # exp(S^T/16)
            nc.scalar.activation(out=pt[:], in_=pss[:], scale=1.0 / 16.0,
                                 func=mybir.ActivationFunctionType.Exp)
            # AV with ones column -> row sums; normalize while copying out
            rst = stage.tile([128, 2, 256], BF16, tag="rst")  # (nchunk, d)
            for nc_ in range(2):
                psr = ps.tile([128, 260], F32, tag="ps")
                for mc in range(2):
                    nc.tensor.matmul(
                        out=psr[:, 0:257],
                        lhsT=pt[:, mc, nc_ * 128:(nc_ + 1) * 128],
                        rhs=x3a[:, b, mc, 0:257],
                        start=(mc == 0), stop=(mc == 1))
                invr = stage.tile([128, 1], F32, tag="invr")
                nc.vector.reciprocal(out=invr[:], in_=psr[:, 256:257])
                nc.vector.tensor_scalar_mul(rst[:, nc_, :], psr[:, 0:256],
                                            invr[:, 0:1])
            # transpose r -> [d, n]
            psrt = ps.tile([128, 2, 2, 128], BF16, tag="ps")  # (dc, nchunk, t)
            for nc_ in range(2):
                for dc in range(2):
                    nc.tensor.transpose(
                        out=psrt[:, dc, nc_, :],
                        in_=rst[:, nc_, dc * 128:(dc + 1) * 128],
                        identity=idn[:])
            rT = stage.tile([128, 2, 256], BF16, tag="rT")
            nc.any.tensor_copy(out=rT[:],
                               in_=psrt[:].rearrange("p dc n t -> p dc (n t)"))
            # layer 4 with extra wb4 column (exact sync-BN E[y*b] term)
            for h in range(2):
                psy = ps.tile([128, 260], F32, tag="ps")
                for dc in range(2):
                    nc.tensor.matmul(
                        out=psy[:, 0:257],
                        lhsT=rT[:, dc, h * 128:(h + 1) * 128],
                        rhs=w4[:, dc, 0:257],
                        start=(dc == 0), stop=(dc == 1))
                nc.vector.bn_stats(out=stats4[h][:, b, :], in_=psy[:, 0:256])
                nc.vector.tensor_copy(out=wsums[:, h, b:b + 1], in_=psy[:, 256:257])
                nc.any.tensor_copy(out=z4[:, b, h, :], in_=psy[:, 0:256])

        # ---- final BN: exact stats all-reduce, apply, relu, store ----------
        arin4 = stats_to_arin(4, stats4, width=6)
        for h in range(2):
            nc.vector.tensor_reduce(out=arin4[:, 4 + h:5 + h], in_=wsums[:, h, :],
                                    axis=mybir.AxisListType.X,
                                    op=mybir.AluOpType.add)
        artot4 = emit_allreduce(4, arin4, 6)
        s4, bst4 = bn_finalize(3, artot4, wterm=artot4[:, 4:6])
        for b in range(B_LOC):
            ost = stage.tile([128, 2, 256], F32, tag="ost")
            orl = stage.tile([128, 2, 256], F32, tag="orl")
            for h in range(2):
                nc.vector.scalar_tensor_tensor(
                    out=ost[:, h, :], in0=z4[:, b, h, :],
                    scalar=s4[:, h:h + 1], in1=bst4[:, h, :],
                    op0=mybir.AluOpType.mult, op1=mybir.AluOpType.add)
            GP_COMPUTE.tensor_scalar_max(orl[:], ost[:], 0.0)
            nc.sync.dma_start(out=out_r[:, b, :, :], in_=orl[:])

    _legalize_waits(nc)
    return nc


_CACHE = {}


def _prep_core_inputs(inputs):
    bf = ml_dtypes.bfloat16
    W = [inputs["W1"], inputs["W2"], inputs["W3"], inputs["W4"]]
    bs = [inputs["b1"], inputs["b2"], inputs["b3"], inputs["b4"]]
    gamma, beta = inputs["gamma"], inputs["beta"]

    w123 = np.zeros((128, 2, 768), dtype=bf)
    for l in range(3):
        for c in range(2):
            w123[:, c, l * 256:(l + 1) * 256] = (
                W[l][:, c * 128:(c + 1) * 128].T.astype(bf))
    w4 = np.zeros((128, 2, 260), dtype=bf)
    wb4 = (W[3].T.astype(np.float64) @ bs[3].astype(np.float64)).astype(np.float32)
    for c in range(2):
        w4[:, c, 0:256] = W[3][:, c * 128:(c + 1) * 128].T.astype(bf)
        w4[:, c, 256] = wb4[c * 128:(c + 1) * 128].astype(bf)
    bb = np.broadcast_to(np.stack(bs, 0)[None], (128, 4, 256)).astype(np.float32)
    bb = np.ascontiguousarray(bb)
    gb = np.zeros((128, 2, 2), dtype=np.float32)
    for h in range(2):
        gb[:, h, 0] = gamma[h * 128:(h + 1) * 128]
        gb[:, h, 1] = beta[h * 128:(h + 1) * 128]
    hc = np.zeros((128, 8), dtype=np.float32)
    for l in range(4):
        hc[:, l] = bs[l].mean(dtype=np.float64)
    for l in range(3):
        hc[:, 4 + l] = (bs[l].astype(np.float64) ** 2).mean() - \
            bs[l].mean(dtype=np.float64) ** 2
    hc[:, 7] = (bs[3].astype(np.float64) ** 2).mean()
    return w123, w4, bb, gb, hc


def kernel(**inputs):
    _install_profile_shim()
    from concourse.bass_utils import run_bass_kernel_spmd

    if "nc" not in _CACHE:
        _CACHE["nc"] = build_program()
    nc = _CACHE["nc"]

    x = np.asarray(inputs["x"], dtype=np.float32)
    w123, w4, bb, gb, hc = _prep_core_inputs(
        {k: np.asarray(v) for k, v in inputs.items()})

    in_maps = []
    for i in range(NCORES):
        xs = np.ascontiguousarray(
            x[i * B_LOC:(i + 1) * B_LOC].reshape(T, 256))
        in_maps.append({"x": xs, "w123": w123, "w4": w4, "bb": bb,
                        "gb": gb, "hc": hc})

    trace = _CACHE.get("trace", False)
    res = run_bass_kernel_spmd(nc, in_maps, list(range(NCORES)), trace=trace)
    _CACHE["last_result"] = res

    out = np.empty((256, 256, 256), dtype=np.float32)
    for i in range(NCORES):
        out[i * B_LOC:(i + 1) * B_LOC] = res.results[i]["out"].reshape(
            B_LOC, 256, 256)
    return out
